# revision 19
# baseline (speedup 1.0000x reference)
"""Trainium2 Bass kernel for ClusteredGraphReconstructor.

Computes, for pos and neg edge sets:
    logit_e = assignments[src_e] @ W[type_e] @ assignments[dst_e] + bias[type_e]
    W[r] = sigmoid(inter_cluster_logits[r]) * clip(sigmoid(log_alpha[r])*1.2 - 0.1, 0, 1)
    loss = mean(softplus(-pos_logits)) + mean(softplus(neg_logits))

Strategy (8-core SPMD, edges sharded across cores; assignments + W replicated):
  Phase 1 (per core): AW[n, r, :] = assignments[n] @ W[r]  -> DRAM table
    [NPAD, R*K] f32, node rows permuted into a DMA-friendly "slot" order.
  Phase 2: edges are grouped by (type r, src window, dst window) on the host
    (dma_gather indices are int16, so tables are windowed to <=32768 rows),
    dealt round-robin to the 8 cores (keeps per-group counts equal so the
    single SPMD program fits all cores).  Per 128*C-edge tile:
      dma_gather s = AW[slot(src), r]   (256B rows)
      dma_gather d = assignments[dst]   (256B rows)
      logit = reduce_k(s*d) + bias_r    (DVE)
      softplus(+-logit) partial sums    (ACT, masked on padded tail slots)
  Host: un-permutes logits, combines the 8x128 partial sums into the scalar
  BCE losses.
"""

import os
import numpy as np

from concourse import bass, bacc, mybir
from concourse import bass_utils, library_config
from concourse.tile import TileContext

F32 = mybir.dt.float32
I16 = mybir.dt.int16
AFT = mybir.ActivationFunctionType
ALU = mybir.AluOpType

LIMIT_A = -0.1
LIMIT_B = 1.1

N_CORES = 8
NB = 1024          # nodes per phase-1 block (8 matmuls x 128 nodes)
WIN = 32768        # gather window (int16 index limit)
T_EDGES = 1024     # edges per phase-2 tile (multiple of 128; dma_gather's
                   # descriptor ring caps one call at 1024 indices)

# stash of the last run's results, for test harnesses
LAST_RESULTS = None


def _roundup(x, m):
    return (x + m - 1) // m * m


def _sigmoid64(x):
    return 1.0 / (1.0 + np.exp(-x.astype(np.float64)))


def _slot_of_node(npad):
    """Permutation mapping node id -> row slot in the AW table.

    Phase 1 emits, per NB-block, an SBUF tile [128p, (NB//128)k * R*K] whose
    natural DRAM layout puts node n = b*NB + k*128 + p at row b*NB + p*(NB//128) + k.
    """
    n = np.arange(npad)
    b, rr = n // NB, n % NB
    return b * NB + (rr % 128) * (NB // 128) + rr // 128


def _tile_sizes(padded):
    """Split a group's padded edge count into tile sizes (multiples of 128)."""
    sizes = [T_EDGES] * (padded // T_EDGES)
    if padded % T_EDGES:
        sizes.append(padded % T_EDGES)
    return sizes


def _idx_layout(idx16):
    """[T] int16 -> flat [128, T//16] SBUF wrapped layout (16p wrap, 8x replicated)."""
    t = idx16.reshape(-1, 16).T          # [16, T//16]
    return np.tile(t, (8, 1)).ravel()    # [128 * T//16]


def _shard_stream(src, dst, et, slot_of, n_w_src, n_w_dst, R):
    """Group + deal edges of one stream. Returns (meta, per_core) where meta is
    the compile-time structure (identical across cores) and per_core holds
    host arrays for each core."""
    slot_src = slot_of[src]
    sw = slot_src // WIN
    dw = dst // WIN
    gid = (et * n_w_src + sw) * n_w_dst + dw
    order = np.argsort(gid, kind="stable")
    gsorted = gid[order]
    n_groups = R * n_w_src * n_w_dst
    starts = np.searchsorted(gsorted, np.arange(n_groups + 1))

    meta = []           # list of dict(r, sw, dw, tiles=[...], padded)
    per_core = [dict(sidx=[], didx=[], mask=[], ids=[], valid=[]) for _ in range(N_CORES)]

    for g in range(n_groups):
        ids = order[starts[g]:starts[g + 1]]
        if len(ids) == 0:
            continue
        r = g // (n_w_src * n_w_dst)
        g_sw = (g // n_w_dst) % n_w_src
        g_dw = g % n_w_dst
        counts = [len(ids[c::N_CORES]) for c in range(N_CORES)]
        # 512-multiple padding keeps the set of distinct tile sizes (and thus
        # Pool registers for num_idxs_reg) small
        padded = _roundup(max(counts), 512)
        tiles = _tile_sizes(padded)
        meta.append(dict(r=r, sw=g_sw, dw=g_dw, tiles=tiles, padded=padded))
        for c in range(N_CORES):
            ids_c = ids[c::N_CORES]
            v = len(ids_c)
            sl = (slot_src[ids_c] - g_sw * WIN).astype(np.int16)
            dl = (dst[ids_c] - g_dw * WIN).astype(np.int16)
            sl = np.concatenate([sl, np.zeros(padded - v, np.int16)])
            dl = np.concatenate([dl, np.zeros(padded - v, np.int16)])
            off = 0
            for tsz in tiles:
                per_core[c]["sidx"].append(_idx_layout(sl[off:off + tsz]))
                per_core[c]["didx"].append(_idx_layout(dl[off:off + tsz]))
                off += tsz
            # mask for the last tile (padding lives only there)
            last = tiles[-1]
            vt = v - (padded - last)        # valid count inside last tile
            cc = last // 128
            m = (np.arange(cc)[None, :] * 128 + np.arange(128)[:, None] < vt)
            per_core[c]["mask"].append(m.astype(np.float32).ravel())
            per_core[c]["ids"].append(ids_c)
            per_core[c]["valid"].append(v)
    return meta, per_core


def _build_program(N, K, R, NPAD, meta_pos, meta_neg, lens):
    """Build the single SPMD Bass program."""
    nc = bacc.Bacc(trn_type="TRN2", num_devices=N_CORES)
    RK = R * K

    asgT_d = nc.declare_dram_parameter("asgT", [K, NPAD], F32, isOutput=False)
    asg_d = nc.declare_dram_parameter("asg", [N, K], F32, isOutput=False)
    wcat_d = nc.declare_dram_parameter("wcat", [K, RK], F32, isOutput=False)
    bias_d = nc.declare_dram_parameter("biasb", [128, R], F32, isOutput=False)
    sidx_d = {s: nc.declare_dram_parameter(f"sidx_{s}", [lens[s]["idx"]], I16, isOutput=False)
              for s in ("pos", "neg")}
    didx_d = {s: nc.declare_dram_parameter(f"didx_{s}", [lens[s]["idx"]], I16, isOutput=False)
              for s in ("pos", "neg")}
    mask_d = {s: nc.declare_dram_parameter(f"mask_{s}", [lens[s]["mask"]], F32, isOutput=False)
              for s in ("pos", "neg")}
    logit_d = {s: nc.declare_dram_parameter(f"logit_{s}", [lens[s]["out"]], F32,
                                            isOutput=True)
               for s in ("pos", "neg")}
    partials_d = nc.declare_dram_parameter("partials", [128, 2], F32, isOutput=True)

    aw_d = nc.dram_tensor("aw", [NPAD, RK], F32)

    n_blocks = NPAD // NB
    kpb = NB // 128                     # matmuls (128-node tiles) per block
    n_tiles = {"pos": sum(len(g["tiles"]) for g in meta_pos),
               "neg": sum(len(g["tiles"]) for g in meta_neg)}
    metas = {"pos": meta_pos, "neg": meta_neg}
    sgn = {"pos": -1.0, "neg": 1.0}

    src_win = [min(WIN, NPAD - w * WIN) for w in range((NPAD + WIN - 1) // WIN)]
    dst_win = [min(WIN, N - w * WIN) for w in range((N + WIN - 1) // WIN)]

    nreg_cache = {}

    def nreg(v):
        if v not in nreg_cache:
            nreg_cache[v] = nc.gpsimd.to_reg(v)
        return nreg_cache[v]

    with TileContext(nc) as tc:
        with (
            tc.tile_pool(name="const", bufs=1) as constp,
            tc.tile_pool(name="asl", bufs=3) as aslp,
            tc.tile_pool(name="awout", bufs=2) as awoutp,
            tc.tile_pool(name="psum", bufs=2, space="PSUM") as psp,
            tc.tile_pool(name="gs", bufs=2) as gsp,
            tc.tile_pool(name="gd", bufs=2) as gdp,
            tc.tile_pool(name="prod", bufs=2) as prodp,
            tc.tile_pool(name="idx", bufs=3) as idxp,
            tc.tile_pool(name="small", bufs=4) as smp,
            tc.tile_pool(name="acc", bufs=1) as accp,
        ):
            nc.gpsimd.load_library(library_config.mlp)
            wcat_sb = constp.tile([K, RK], F32)
            nc.sync.dma_start(out=wcat_sb[:], in_=wcat_d[:, :])
            bias_sb = constp.tile([128, R], F32)
            nc.sync.dma_start(out=bias_sb[:], in_=bias_d[:, :])

            # ---- Phase 1: AW[n] = asg[n] @ W (all R at once) ----
            for it in range(n_blocks):
                a_sl = aslp.tile([K, NB], F32)
                nc.sync.dma_start(out=a_sl[:], in_=asgT_d[:, it * NB:(it + 1) * NB])
                ps = psp.tile([128, kpb * RK], F32)
                for k in range(kpb):
                    nc.tensor.matmul(
                        ps[:, k * RK:(k + 1) * RK],
                        lhsT=a_sl[:, k * 128:(k + 1) * 128],
                        rhs=wcat_sb[:],
                        start=True, stop=True,
                    )
                ot = awoutp.tile([128, kpb * RK], F32)
                half = kpb * RK // 2
                nc.vector.tensor_copy(ot[:, :half], ps[:, :half])
                nc.scalar.copy(ot[:, half:], ps[:, half:])
                dview = aw_d[it * NB:(it + 1) * NB, :].rearrange(
                    "(p k) f -> p (k f)", p=128)
                nc.sync.dma_start(out=dview, in_=ot[:])

            tc.strict_bb_all_engine_barrier()

            # ---- Phase 2: gather + dot per edge tile ----
            acc_sb = {s: accp.tile([128, max(1, n_tiles[s])], F32,
                                   name=f"acc_{s}", tag=f"acc_{s}")
                      for s in metas}
            for s in ("pos", "neg"):
                ioff = ooff = moff = 0
                ti = 0
                mi = 0
                for g in metas[s]:
                    r = g["r"]
                    s_ap = aw_d[g["sw"] * WIN:g["sw"] * WIN + src_win[g["sw"]],
                                r * K:(r + 1) * K]
                    d_ap = asg_d[g["dw"] * WIN:g["dw"] * WIN + dst_win[g["dw"]], :]
                    for tix, tsz in enumerate(g["tiles"]):
                        cc = tsz // 128
                        si = idxp.tile([128, T_EDGES // 16], I16, tag="si")
                        di = idxp.tile([128, T_EDGES // 16], I16, tag="di")
                        nc.sync.dma_start(
                            out=si[:, :tsz // 16],
                            in_=sidx_d[s][ioff:ioff + tsz * 8].rearrange(
                                "(p f) -> p f", p=128))
                        nc.sync.dma_start(
                            out=di[:, :tsz // 16],
                            in_=didx_d[s][ioff:ioff + tsz * 8].rearrange(
                                "(p f) -> p f", p=128))
                        ioff += tsz * 8

                        s_t = gsp.tile([128, T_EDGES // 128 * K], F32, tag="s_t")
                        d_t = gdp.tile([128, T_EDGES // 128 * K], F32, tag="d_t")
                        nc.gpsimd.dma_gather(
                            out_ap=s_t[:, :cc * K].rearrange("p (c k) -> p c k", k=K),
                            in_ap=s_ap, idxs_ap=si[:, :tsz // 16],
                            num_idxs=tsz, num_idxs_reg=nreg(tsz),
                            elem_size=K, elem_step=RK,
                        )
                        nc.gpsimd.dma_gather(
                            out_ap=d_t[:, :cc * K].rearrange("p (c k) -> p c k", k=K),
                            in_ap=d_ap, idxs_ap=di[:, :tsz // 16],
                            num_idxs=tsz, num_idxs_reg=nreg(tsz),
                            elem_size=K,
                        )
                        prod = prodp.tile([128, T_EDGES // 128 * K], F32, tag="prod")
                        nc.vector.tensor_mul(prod[:, :cc * K], s_t[:, :cc * K],
                                             d_t[:, :cc * K])
                        logit = smp.tile([128, T_EDGES // 128], F32, tag="logit")
                        nc.vector.reduce_sum(
                            logit[:, :cc],
                            prod[:, :cc * K].rearrange("p (c k) -> p c k", k=K),
                            axis=mybir.AxisListType.X)
                        nc.vector.tensor_scalar_add(logit[:, :cc], logit[:, :cc],
                                                    bias_sb[:, r:r + 1])
                        nc.sync.dma_start(
                            out=logit_d[s][ooff:ooff + tsz].rearrange(
                                "(p f) -> p f", p=128),
                            in_=logit[:, :cc])
                        ooff += tsz

                        # stable softplus(sgn*logit) = relu(a) + ln(1+exp(-|a|))
                        a_ = smp.tile([128, T_EDGES // 128], F32, tag="a_")
                        nb = smp.tile([128, T_EDGES // 128], F32, tag="nb")
                        nc.vector.tensor_single_scalar(
                            a_[:, :cc], logit[:, :cc], sgn[s], op=ALU.mult)
                        nc.vector.tensor_single_scalar(
                            nb[:, :cc], logit[:, :cc], -sgn[s], op=ALU.mult)
                        nc.vector.tensor_tensor(nb[:, :cc], a_[:, :cc], nb[:, :cc],
                                                op=ALU.min)
                        nc.scalar.activation(nb[:, :cc], nb[:, :cc], AFT.Exp)
                        nc.scalar.activation(nb[:, :cc], nb[:, :cc], AFT.Ln,
                                             bias=1.0)
                        nc.vector.tensor_single_scalar(
                            a_[:, :cc], a_[:, :cc], 0.0, op=ALU.max)
                        sp = smp.tile([128, T_EDGES // 128], F32, tag="sp")
                        nc.vector.tensor_add(sp[:, :cc], a_[:, :cc], nb[:, :cc])
                        if tix == len(g["tiles"]) - 1:
                            mk = idxp.tile([128, T_EDGES // 128], F32, tag="mk")
                            nc.sync.dma_start(
                                out=mk[:, :cc],
                                in_=mask_d[s][moff:moff + tsz].rearrange(
                                    "(p f) -> p f", p=128))
                            moff += tsz
                            spm = smp.tile([128, T_EDGES // 128], F32, tag="spm")
                            nc.vector.tensor_mul(spm[:, :cc], sp[:, :cc],
                                                 mk[:, :cc])
                            sp = spm
                            mi += 1
                        nc.vector.reduce_sum(acc_sb[s][:, ti:ti + 1], sp[:, :cc],
                                             axis=mybir.AxisListType.X)
                        ti += 1

            pk = smp.tile([128, 2], F32, tag="pk")
            for col, s in enumerate(("pos", "neg")):
                if n_tiles[s] > 0:
                    nc.vector.reduce_sum(pk[:, col:col + 1], acc_sb[s][:],
                                         axis=mybir.AxisListType.X)
                else:
                    nc.vector.memset(pk[:, col:col + 1], 0.0)
            nc.sync.dma_start(out=partials_d[:, :], in_=pk[:])

    nc.compile()
    return nc


def kernel(assignments, inter_cluster_logits, log_alpha, absent_bias,
           edge_index, edge_type, neg_edge_index, neg_edge_type):
    global LAST_RESULTS
    N, K = assignments.shape
    R = inter_cluster_logits.shape[0]
    assert K == 64 and WIN % NB == 0

    # host-side weight prep (tiny: [R, K, K])
    gate = np.clip(_sigmoid64(log_alpha) * (LIMIT_B - LIMIT_A) + LIMIT_A, 0.0, 1.0)
    W = (_sigmoid64(inter_cluster_logits) * gate).astype(np.float32)   # [R, K, K]
    wcat = np.ascontiguousarray(W.transpose(1, 0, 2).reshape(K, R * K))
    bias_f32 = np.asarray(absent_bias, np.float32)
    biasb = np.tile(bias_f32[None, :], (128, 1))

    NPAD = _roundup(N, NB)
    asg_f32 = np.asarray(assignments, np.float32)
    asgT = np.zeros((K, NPAD), np.float32)
    asgT[:, :N] = asg_f32.T

    slot_of = _slot_of_node(NPAD)
    n_w_src = (NPAD + WIN - 1) // WIN
    n_w_dst = (N + WIN - 1) // WIN

    streams = {}
    per_cores = {}
    for name, (ei, et) in (("pos", (edge_index, edge_type)),
                           ("neg", (neg_edge_index, neg_edge_type))):
        src = np.asarray(ei[0], np.int64)
        dst = np.asarray(ei[1], np.int64)
        streams[name], per_cores[name] = _shard_stream(
            src, dst, np.asarray(et, np.int64), slot_of, n_w_src, n_w_dst, R)

    lens = {}
    for s in ("pos", "neg"):
        padded_total = sum(g["padded"] for g in streams[s])
        lens[s] = dict(idx=padded_total * 8, out=padded_total,
                       mask=sum(g["tiles"][-1] for g in streams[s]))
        for k_ in lens[s]:
            lens[s][k_] = max(lens[s][k_], 128)  # avoid zero-size tensors

    nc = _build_program(N, K, R, NPAD, streams["pos"], streams["neg"], lens)

    in_maps = []
    for c in range(N_CORES):
        m = dict(asgT=asgT, asg=asg_f32, wcat=wcat, biasb=biasb)
        for s in ("pos", "neg"):
            pc = per_cores[s][c]
            for key, arrs, dt, ln in (
                (f"sidx_{s}", pc["sidx"], np.int16, lens[s]["idx"]),
                (f"didx_{s}", pc["didx"], np.int16, lens[s]["idx"]),
                (f"mask_{s}", pc["mask"], np.float32, lens[s]["mask"]),
            ):
                flat = (np.concatenate(arrs) if arrs else np.zeros(0, dt)).astype(dt)
                if len(flat) < ln:
                    flat = np.concatenate([flat, np.zeros(ln - len(flat), dt)])
                m[key] = flat
        in_maps.append(m)

    os.environ["BASS_NEVER_TRACE"] = "1"   # bass_utils' own axon trace path
    # is broken in this image (antenv.axon_hooks missing); we capture NTFFs
    # ourselves below when BASSGNN_TRACE is set.
    hook = None
    if os.environ.get("BASSGNN_TRACE"):
        try:
            import shutil
            from trn_agent_boot.trn_boot import _ntff_profile_via_ctypes
            outdir = os.environ.get("BASSGNN_TRACE_DIR", "/tmp/bassgnn_ntff")
            shutil.rmtree(outdir, ignore_errors=True)
            os.makedirs(outdir, exist_ok=True)
            hook = _ntff_profile_via_ctypes("/opt/axon/libaxon_pjrt.so")
        except Exception:
            hook = None
    if hook is not None:
        trace_cores = os.environ.get("BASSGNN_TRACE_CORES", "0")
        ids = [int(x) for x in trace_cores.split(",")]
        with hook(outdir, ids):
            res = bass_utils.run_bass_kernel_spmd(
                nc, in_maps, list(range(N_CORES)), trace=False)
    else:
        res = bass_utils.run_bass_kernel_spmd(
            nc, in_maps, list(range(N_CORES)), trace=False)
    LAST_RESULTS = res

    # ---- host-side unpermute + loss combine ----
    E = {"pos": edge_index.shape[1], "neg": neg_edge_index.shape[1]}
    logits_full = {s: np.zeros(E[s], np.float32) for s in ("pos", "neg")}
    sums = {"pos": 0.0, "neg": 0.0}
    for c in range(N_CORES):
        out = res.results[c]
        part = out["partials"]
        sums["pos"] += float(part[:, 0].astype(np.float64).sum())
        sums["neg"] += float(part[:, 1].astype(np.float64).sum())
        for s in ("pos", "neg"):
            arr = out[f"logit_{s}"]
            pc = per_cores[s][c]
            off = 0
            gi = 0
            for g in streams[s]:
                vals = []
                for tsz in g["tiles"]:
                    block = arr[off:off + tsz].reshape(128, tsz // 128)
                    vals.append(block.T.ravel())
                    off += tsz
                vals = np.concatenate(vals)
                ids_c = pc["ids"][gi]
                logits_full[s][ids_c] = vals[:pc["valid"][gi]]
                gi += 1

    pos_loss = sums["pos"] / E["pos"]
    neg_loss = sums["neg"] / E["neg"]
    recon = np.float32(pos_loss + neg_loss)
    return recon, logits_full["pos"], logits_full["neg"]


# revision 20
# speedup vs baseline: 1.7813x; 1.7813x over previous
"""Trainium2 Bass kernel for ClusteredGraphReconstructor.

Computes, for pos and neg edge sets:
    logit_e = assignments[src_e] @ W[type_e] @ assignments[dst_e] + bias[type_e]
    W[r] = sigmoid(inter_cluster_logits[r]) * clip(sigmoid(log_alpha[r])*1.2 - 0.1, 0, 1)
    loss = mean(softplus(-pos_logits)) + mean(softplus(neg_logits))

Strategy (8-core SPMD, edges sharded across cores; assignments + W replicated):
  Phase 1 (per core): AW[n, r, :] = assignments[n] @ W[r]  -> DRAM table
    [NPAD, R*K] f32, node rows permuted into a DMA-friendly "slot" order.
  Phase 2: edges are grouped by (type r, src window, dst window) on the host
    (dma_gather indices are int16, so tables are windowed to <=32768 rows),
    dealt round-robin to the 8 cores (keeps per-group counts equal so the
    single SPMD program fits all cores).  Per 128*C-edge tile:
      dma_gather s = AW[slot(src), r]   (256B rows)
      dma_gather d = assignments[dst]   (256B rows)
      logit = reduce_k(s*d) + bias_r    (DVE)
      softplus(+-logit) partial sums    (ACT, masked on padded tail slots)
  Host: un-permutes logits, combines the 8x128 partial sums into the scalar
  BCE losses.
"""

import os
import numpy as np

from concourse import bass, bacc, mybir
from concourse import bass_utils, library_config
from concourse.tile import TileContext

F32 = mybir.dt.float32
I16 = mybir.dt.int16
AFT = mybir.ActivationFunctionType
ALU = mybir.AluOpType

LIMIT_A = -0.1
LIMIT_B = 1.1

N_CORES = 8
NB = 1024          # nodes per phase-1 block (8 matmuls x 128 nodes)
WIN = 32768        # gather window (int16 index limit)
T_EDGES = 1024     # edges per phase-2 tile (multiple of 128; dma_gather's
                   # descriptor ring caps one call at 1024 indices)

# stash of the last run's results, for test harnesses
LAST_RESULTS = None


def _roundup(x, m):
    return (x + m - 1) // m * m


def _sigmoid64(x):
    return 1.0 / (1.0 + np.exp(-x.astype(np.float64)))


def _slot_of_node(npad):
    """Permutation mapping node id -> row slot in the AW table.

    Phase 1 emits, per NB-block, an SBUF tile [128p, (NB//128)k * R*K] whose
    natural DRAM layout puts node n = b*NB + k*128 + p at row b*NB + p*(NB//128) + k.
    """
    n = np.arange(npad)
    b, rr = n // NB, n % NB
    return b * NB + (rr % 128) * (NB // 128) + rr // 128


def _tile_sizes(padded):
    """Split a group's padded edge count into tile sizes (multiples of 128)."""
    sizes = [T_EDGES] * (padded // T_EDGES)
    if padded % T_EDGES:
        sizes.append(padded % T_EDGES)
    return sizes


def _idx_layout(idx16):
    """[T] int16 -> flat [128, T//16] SBUF wrapped layout (16p wrap, 8x replicated)."""
    t = idx16.reshape(-1, 16).T          # [16, T//16]
    return np.tile(t, (8, 1)).ravel()    # [128 * T//16]


def _shard_stream(src, dst, et, slot_of, n_w_src, n_w_dst, R):
    """Group + deal edges of one stream. Returns (meta, per_core) where meta is
    the compile-time structure (identical across cores) and per_core holds
    host arrays for each core."""
    slot_src = slot_of[src]
    sw = slot_src // WIN
    dw = dst // WIN
    gid = (et * n_w_src + sw) * n_w_dst + dw
    order = np.argsort(gid, kind="stable")
    gsorted = gid[order]
    n_groups = R * n_w_src * n_w_dst
    starts = np.searchsorted(gsorted, np.arange(n_groups + 1))

    meta = []           # list of dict(r, sw, dw, tiles=[...], padded)
    per_core = [dict(sidx=[], didx=[], mask=[], ids=[], valid=[]) for _ in range(N_CORES)]

    for g in range(n_groups):
        ids = order[starts[g]:starts[g + 1]]
        if len(ids) == 0:
            continue
        r = g // (n_w_src * n_w_dst)
        g_sw = (g // n_w_dst) % n_w_src
        g_dw = g % n_w_dst
        counts = [len(ids[c::N_CORES]) for c in range(N_CORES)]
        # 512-multiple padding keeps the set of distinct tile sizes (and thus
        # Pool registers for num_idxs_reg) small
        padded = _roundup(max(counts), 512)
        tiles = _tile_sizes(padded)
        meta.append(dict(r=r, sw=g_sw, dw=g_dw, tiles=tiles, padded=padded))
        for c in range(N_CORES):
            ids_c = ids[c::N_CORES]
            v = len(ids_c)
            sl = (slot_src[ids_c] - g_sw * WIN).astype(np.int16)
            dl = (dst[ids_c] - g_dw * WIN).astype(np.int16)
            sl = np.concatenate([sl, np.zeros(padded - v, np.int16)])
            dl = np.concatenate([dl, np.zeros(padded - v, np.int16)])
            off = 0
            for tsz in tiles:
                per_core[c]["sidx"].append(_idx_layout(sl[off:off + tsz]))
                per_core[c]["didx"].append(_idx_layout(dl[off:off + tsz]))
                off += tsz
            # mask for the last tile (padding lives only there)
            last = tiles[-1]
            vt = v - (padded - last)        # valid count inside last tile
            cc = last // 128
            m = (np.arange(cc)[None, :] * 128 + np.arange(128)[:, None] < vt)
            per_core[c]["mask"].append(m.astype(np.float32).ravel())
            per_core[c]["ids"].append(ids_c)
            per_core[c]["valid"].append(v)
    return meta, per_core


def _build_program(N, K, R, NPAD, meta_pos, meta_neg, lens):
    """Build the single SPMD Bass program."""
    nc = bacc.Bacc(trn_type="TRN2", num_devices=N_CORES, num_swdge_queues=2)
    RK = R * K

    asgT_d = nc.declare_dram_parameter("asgT", [K, NPAD], F32, isOutput=False)
    asg_d = nc.declare_dram_parameter("asg", [N, K], F32, isOutput=False)
    wcat_d = nc.declare_dram_parameter("wcat", [K, RK], F32, isOutput=False)
    bias_d = nc.declare_dram_parameter("biasb", [128, R], F32, isOutput=False)
    sidx_d = {s: nc.declare_dram_parameter(f"sidx_{s}", [lens[s]["idx"]], I16, isOutput=False)
              for s in ("pos", "neg")}
    didx_d = {s: nc.declare_dram_parameter(f"didx_{s}", [lens[s]["idx"]], I16, isOutput=False)
              for s in ("pos", "neg")}
    mask_d = {s: nc.declare_dram_parameter(f"mask_{s}", [lens[s]["mask"]], F32, isOutput=False)
              for s in ("pos", "neg")}
    logit_d = {s: nc.declare_dram_parameter(f"logit_{s}", [lens[s]["out"]], F32,
                                            isOutput=True)
               for s in ("pos", "neg")}
    partials_d = nc.declare_dram_parameter("partials", [128, 2], F32, isOutput=True)

    aw_d = nc.dram_tensor("aw", [NPAD, RK], F32)

    n_blocks = NPAD // NB
    kpb = NB // 128                     # matmuls (128-node tiles) per block
    n_tiles = {"pos": sum(len(g["tiles"]) for g in meta_pos),
               "neg": sum(len(g["tiles"]) for g in meta_neg)}
    metas = {"pos": meta_pos, "neg": meta_neg}
    sgn = {"pos": -1.0, "neg": 1.0}

    src_win = [min(WIN, NPAD - w * WIN) for w in range((NPAD + WIN - 1) // WIN)]
    dst_win = [min(WIN, N - w * WIN) for w in range((N + WIN - 1) // WIN)]

    nreg_cache = {}

    def nreg(v):
        if v not in nreg_cache:
            nreg_cache[v] = nc.gpsimd.to_reg(v)
        return nreg_cache[v]

    with TileContext(nc) as tc:
        with (
            tc.tile_pool(name="const", bufs=1) as constp,
            tc.tile_pool(name="asl", bufs=3) as aslp,
            tc.tile_pool(name="awout", bufs=2) as awoutp,
            tc.tile_pool(name="psum", bufs=2, space="PSUM") as psp,
            tc.tile_pool(name="gs", bufs=2) as gsp,
            tc.tile_pool(name="gd", bufs=2) as gdp,
            tc.tile_pool(name="prod", bufs=2) as prodp,
            tc.tile_pool(name="idx", bufs=3) as idxp,
            tc.tile_pool(name="small", bufs=4) as smp,
            tc.tile_pool(name="acc", bufs=1) as accp,
        ):
            nc.gpsimd.load_library(library_config.mlp)
            wcat_sb = constp.tile([K, RK], F32)
            nc.sync.dma_start(out=wcat_sb[:], in_=wcat_d[:, :])
            bias_sb = constp.tile([128, R], F32)
            nc.sync.dma_start(out=bias_sb[:], in_=bias_d[:, :])

            # ---- Phase 1: AW[n] = asg[n] @ W (all R at once) ----
            for it in range(n_blocks):
                a_sl = aslp.tile([K, NB], F32)
                nc.sync.dma_start(out=a_sl[:], in_=asgT_d[:, it * NB:(it + 1) * NB])
                ps = psp.tile([128, kpb * RK], F32)
                for k in range(kpb):
                    nc.tensor.matmul(
                        ps[:, k * RK:(k + 1) * RK],
                        lhsT=a_sl[:, k * 128:(k + 1) * 128],
                        rhs=wcat_sb[:],
                        start=True, stop=True,
                    )
                ot = awoutp.tile([128, kpb * RK], F32)
                half = kpb * RK // 2
                nc.vector.tensor_copy(ot[:, :half], ps[:, :half])
                nc.scalar.copy(ot[:, half:], ps[:, half:])
                dview = aw_d[it * NB:(it + 1) * NB, :].rearrange(
                    "(p k) f -> p (k f)", p=128)
                nc.sync.dma_start(out=dview, in_=ot[:])

            tc.strict_bb_all_engine_barrier()

            # ---- Phase 2: gather + dot per edge tile ----
            acc_sb = {s: accp.tile([128, max(1, n_tiles[s])], F32,
                                   name=f"acc_{s}", tag=f"acc_{s}")
                      for s in metas}
            for s in ("pos", "neg"):
                ioff = ooff = moff = 0
                ti = 0
                mi = 0
                for g in metas[s]:
                    r = g["r"]
                    s_ap = aw_d[g["sw"] * WIN:g["sw"] * WIN + src_win[g["sw"]],
                                r * K:(r + 1) * K]
                    d_ap = asg_d[g["dw"] * WIN:g["dw"] * WIN + dst_win[g["dw"]], :]
                    for tix, tsz in enumerate(g["tiles"]):
                        cc = tsz // 128
                        si = idxp.tile([128, T_EDGES // 16], I16, tag="si")
                        di = idxp.tile([128, T_EDGES // 16], I16, tag="di")
                        nc.sync.dma_start(
                            out=si[:, :tsz // 16],
                            in_=sidx_d[s][ioff:ioff + tsz * 8].rearrange(
                                "(p f) -> p f", p=128))
                        nc.sync.dma_start(
                            out=di[:, :tsz // 16],
                            in_=didx_d[s][ioff:ioff + tsz * 8].rearrange(
                                "(p f) -> p f", p=128))
                        ioff += tsz * 8

                        s_t = gsp.tile([128, T_EDGES // 128 * K], F32, tag="s_t")
                        d_t = gdp.tile([128, T_EDGES // 128 * K], F32, tag="d_t")
                        nc.gpsimd.dma_gather(
                            out_ap=s_t[:, :cc * K].rearrange("p (c k) -> p c k", k=K),
                            in_ap=s_ap, idxs_ap=si[:, :tsz // 16],
                            num_idxs=tsz, num_idxs_reg=nreg(tsz),
                            elem_size=K, elem_step=RK, queue_num=0,
                        )
                        nc.gpsimd.dma_gather(
                            out_ap=d_t[:, :cc * K].rearrange("p (c k) -> p c k", k=K),
                            in_ap=d_ap, idxs_ap=di[:, :tsz // 16],
                            num_idxs=tsz, num_idxs_reg=nreg(tsz),
                            elem_size=K, queue_num=1,
                        )
                        prod = prodp.tile([128, T_EDGES // 128 * K], F32, tag="prod")
                        nc.vector.tensor_mul(prod[:, :cc * K], s_t[:, :cc * K],
                                             d_t[:, :cc * K])
                        logit = smp.tile([128, T_EDGES // 128], F32, tag="logit")
                        nc.vector.reduce_sum(
                            logit[:, :cc],
                            prod[:, :cc * K].rearrange("p (c k) -> p c k", k=K),
                            axis=mybir.AxisListType.X)
                        nc.vector.tensor_scalar_add(logit[:, :cc], logit[:, :cc],
                                                    bias_sb[:, r:r + 1])
                        nc.sync.dma_start(
                            out=logit_d[s][ooff:ooff + tsz].rearrange(
                                "(p f) -> p f", p=128),
                            in_=logit[:, :cc])
                        ooff += tsz

                        # stable softplus(sgn*logit) = relu(a) + ln(1+exp(-|a|))
                        a_ = smp.tile([128, T_EDGES // 128], F32, tag="a_")
                        nb = smp.tile([128, T_EDGES // 128], F32, tag="nb")
                        nc.vector.tensor_single_scalar(
                            a_[:, :cc], logit[:, :cc], sgn[s], op=ALU.mult)
                        nc.vector.tensor_single_scalar(
                            nb[:, :cc], logit[:, :cc], -sgn[s], op=ALU.mult)
                        nc.vector.tensor_tensor(nb[:, :cc], a_[:, :cc], nb[:, :cc],
                                                op=ALU.min)
                        nc.scalar.activation(nb[:, :cc], nb[:, :cc], AFT.Exp)
                        nc.scalar.activation(nb[:, :cc], nb[:, :cc], AFT.Ln,
                                             bias=1.0)
                        nc.vector.tensor_single_scalar(
                            a_[:, :cc], a_[:, :cc], 0.0, op=ALU.max)
                        sp = smp.tile([128, T_EDGES // 128], F32, tag="sp")
                        nc.vector.tensor_add(sp[:, :cc], a_[:, :cc], nb[:, :cc])
                        if tix == len(g["tiles"]) - 1:
                            mk = idxp.tile([128, T_EDGES // 128], F32, tag="mk")
                            nc.sync.dma_start(
                                out=mk[:, :cc],
                                in_=mask_d[s][moff:moff + tsz].rearrange(
                                    "(p f) -> p f", p=128))
                            moff += tsz
                            spm = smp.tile([128, T_EDGES // 128], F32, tag="spm")
                            nc.vector.tensor_mul(spm[:, :cc], sp[:, :cc],
                                                 mk[:, :cc])
                            sp = spm
                            mi += 1
                        nc.vector.reduce_sum(acc_sb[s][:, ti:ti + 1], sp[:, :cc],
                                             axis=mybir.AxisListType.X)
                        ti += 1

            pk = smp.tile([128, 2], F32, tag="pk")
            for col, s in enumerate(("pos", "neg")):
                if n_tiles[s] > 0:
                    nc.vector.reduce_sum(pk[:, col:col + 1], acc_sb[s][:],
                                         axis=mybir.AxisListType.X)
                else:
                    nc.vector.memset(pk[:, col:col + 1], 0.0)
            nc.sync.dma_start(out=partials_d[:, :], in_=pk[:])

    nc.compile()
    return nc


def kernel(assignments, inter_cluster_logits, log_alpha, absent_bias,
           edge_index, edge_type, neg_edge_index, neg_edge_type):
    global LAST_RESULTS
    N, K = assignments.shape
    R = inter_cluster_logits.shape[0]
    assert K == 64 and WIN % NB == 0

    # host-side weight prep (tiny: [R, K, K])
    gate = np.clip(_sigmoid64(log_alpha) * (LIMIT_B - LIMIT_A) + LIMIT_A, 0.0, 1.0)
    W = (_sigmoid64(inter_cluster_logits) * gate).astype(np.float32)   # [R, K, K]
    wcat = np.ascontiguousarray(W.transpose(1, 0, 2).reshape(K, R * K))
    bias_f32 = np.asarray(absent_bias, np.float32)
    biasb = np.tile(bias_f32[None, :], (128, 1))

    NPAD = _roundup(N, NB)
    asg_f32 = np.asarray(assignments, np.float32)
    asgT = np.zeros((K, NPAD), np.float32)
    asgT[:, :N] = asg_f32.T

    slot_of = _slot_of_node(NPAD)
    n_w_src = (NPAD + WIN - 1) // WIN
    n_w_dst = (N + WIN - 1) // WIN

    streams = {}
    per_cores = {}
    for name, (ei, et) in (("pos", (edge_index, edge_type)),
                           ("neg", (neg_edge_index, neg_edge_type))):
        src = np.asarray(ei[0], np.int64)
        dst = np.asarray(ei[1], np.int64)
        streams[name], per_cores[name] = _shard_stream(
            src, dst, np.asarray(et, np.int64), slot_of, n_w_src, n_w_dst, R)

    lens = {}
    for s in ("pos", "neg"):
        padded_total = sum(g["padded"] for g in streams[s])
        lens[s] = dict(idx=padded_total * 8, out=padded_total,
                       mask=sum(g["tiles"][-1] for g in streams[s]))
        for k_ in lens[s]:
            lens[s][k_] = max(lens[s][k_], 128)  # avoid zero-size tensors

    nc = _build_program(N, K, R, NPAD, streams["pos"], streams["neg"], lens)

    in_maps = []
    for c in range(N_CORES):
        m = dict(asgT=asgT, asg=asg_f32, wcat=wcat, biasb=biasb)
        for s in ("pos", "neg"):
            pc = per_cores[s][c]
            for key, arrs, dt, ln in (
                (f"sidx_{s}", pc["sidx"], np.int16, lens[s]["idx"]),
                (f"didx_{s}", pc["didx"], np.int16, lens[s]["idx"]),
                (f"mask_{s}", pc["mask"], np.float32, lens[s]["mask"]),
            ):
                flat = (np.concatenate(arrs) if arrs else np.zeros(0, dt)).astype(dt)
                if len(flat) < ln:
                    flat = np.concatenate([flat, np.zeros(ln - len(flat), dt)])
                m[key] = flat
        in_maps.append(m)

    os.environ["BASS_NEVER_TRACE"] = "1"   # bass_utils' own axon trace path
    # is broken in this image (antenv.axon_hooks missing); we capture NTFFs
    # ourselves below when BASSGNN_TRACE is set.
    hook = None
    if os.environ.get("BASSGNN_TRACE"):
        try:
            import shutil
            from trn_agent_boot.trn_boot import _ntff_profile_via_ctypes
            outdir = os.environ.get("BASSGNN_TRACE_DIR", "/tmp/bassgnn_ntff")
            shutil.rmtree(outdir, ignore_errors=True)
            os.makedirs(outdir, exist_ok=True)
            hook = _ntff_profile_via_ctypes("/opt/axon/libaxon_pjrt.so")
        except Exception:
            hook = None
    if hook is not None:
        trace_cores = os.environ.get("BASSGNN_TRACE_CORES", "0")
        ids = [int(x) for x in trace_cores.split(",")]
        with hook(outdir, ids):
            res = bass_utils.run_bass_kernel_spmd(
                nc, in_maps, list(range(N_CORES)), trace=False)
    else:
        res = bass_utils.run_bass_kernel_spmd(
            nc, in_maps, list(range(N_CORES)), trace=False)
    LAST_RESULTS = res

    # ---- host-side unpermute + loss combine ----
    E = {"pos": edge_index.shape[1], "neg": neg_edge_index.shape[1]}
    logits_full = {s: np.zeros(E[s], np.float32) for s in ("pos", "neg")}
    sums = {"pos": 0.0, "neg": 0.0}
    for c in range(N_CORES):
        out = res.results[c]
        part = out["partials"]
        sums["pos"] += float(part[:, 0].astype(np.float64).sum())
        sums["neg"] += float(part[:, 1].astype(np.float64).sum())
        for s in ("pos", "neg"):
            arr = out[f"logit_{s}"]
            pc = per_cores[s][c]
            off = 0
            gi = 0
            for g in streams[s]:
                vals = []
                for tsz in g["tiles"]:
                    block = arr[off:off + tsz].reshape(128, tsz // 128)
                    vals.append(block.T.ravel())
                    off += tsz
                vals = np.concatenate(vals)
                ids_c = pc["ids"][gi]
                logits_full[s][ids_c] = vals[:pc["valid"][gi]]
                gi += 1

    pos_loss = sums["pos"] / E["pos"]
    neg_loss = sums["neg"] / E["neg"]
    recon = np.float32(pos_loss + neg_loss)
    return recon, logits_full["pos"], logits_full["neg"]
